# revision 81
# baseline (speedup 1.0000x reference)
"""Trainium2 Bass kernel for nn_Attention_80917183857290.

Multi-head causal attention (B=2, S=2048, D=1024, H=32, HD=32) with
SSMax-style per-query log-length score scaling, run SPMD on 8 NeuronCores.

Sharding: core c -> batch b = c // 4, head-group g2 = c % 4 (8 heads each).
Per core:
  - projections q,k (transposed layout [head_dim rows, seq]) and v, from
    bf16 x^T / bf16 weights (psum accumulates fp32; q/k kept f32r on-chip);
    a fused startup unit interleaves the first q/k/v matmuls k-major so they
    trail the x chunk DMAs
  - scores^T[k, q] per head via row-tiled K=32 matmuls (4 heads packed into
    the PE array via tile_position; two 2-bank PSUM tiles per k-tile entry,
    double-buffered)
  - probs = exp(scores) * emask (bf16), with the emask multiply narrowed to
    the partial-mask column window; fully-masked leading columns skipped (zq)
  - out^T[hd, q] and softmax denominators in one pass: V is ones-augmented
    and zero-padded to a 64-wide slot ([v_h | 1 | 0...]) so the two PV
    matmuls per bank cover all 128 partitions; PV accumulates into a
    per-(group, chunk) 2-bank PSUM tile with start=True on the first entry
    (no memset). PV matmuls trail the entry stream by one entry so the
    next entry's scores never wait on exp/PSUM WAR hazards.
  - compaction (deferred into the next group's entry stream): DVE copies
    pv/dn to SBUF, host-built permutation matmuls compact 4 heads +
    replicated denominators into a fresh 2-bank tile; att = pv * recip(dn)
    -> bf16; partial = att^T.T @ wo via PE. The final chunk's compaction is
    split into column halves interleaved with the last wo units, whose
    kk=0 matmuls are pre-started.
  - host sums the 4 partial outputs per batch.

PSUM budget (8 banks): pv tag [P,1024]x1 + sc tag [P,1024]x2 (scores double
buffer, also reused for the compaction output) + pj tag [P,512]x2
(projection / wo accumulators) = 16KB per partition exactly.

Emission is resource-tracked: projection units are required lazily
just-in-time per entry and otherwise queued as one-shot filler units popped
between attention entries (wo units split in half for finer granularity),
so the PE never starves while ACT runs exp; wo filler units are deferred
toward the later (larger) attention chunks. In the final group the pv/dn
SBUF copies are split across DVE and ACT and the first-half columns are
copied out two entries early (the remaining causal entries only touch
columns >= 256), shortening the tail chain. TimelineSim: 203023 ns
(baseline 258710 ns).
"""

import math
from collections import deque

import numpy as np
import ml_dtypes

B, S, D, H = 2, 2048, 1024, 32
HD = D // H  # 32
P = 128
QC = 512  # q-chunk (PSUM bank free size, fp32)
NQC = S // QC  # 4
NKT = S // P  # 16
NCORES = 8

_GRAPH_CACHE: dict = {}


def _build_graph(plans, nt, reps=1):
    """Build the per-core Bass graph.

    plans: tuple over qc (4) of tuple of (kt, mask_idx, zq, w1) entries;
      mask_idx -1 means no mask multiply; >=0 indexes the packed emask tile
      array, multiplied over columns [zq, w1).
    nt: number of packed [128, 512] bf16 exp-mask tiles (>= 1).
    """
    import concourse.mybir as mybir
    from concourse import bacc
    from concourse.tile import TileContext

    f32 = mybir.dt.float32
    f32r = mybir.dt.float32r
    bf16 = mybir.dt.bfloat16
    EXP = mybir.ActivationFunctionType.Exp
    MULT = mybir.AluOpType.mult

    nc = bacc.Bacc()

    xT = nc.declare_dram_parameter("xT", [D, S], bf16, isOutput=False)
    aq = nc.declare_dram_parameter("aq", [D, 256], bf16, isOutput=False)
    ak = nc.declare_dram_parameter("ak", [D, 256], bf16, isOutput=False)
    av = nc.declare_dram_parameter("av", [D, 256], bf16, isOutput=False)
    wor = nc.declare_dram_parameter("wor", [256, D], bf16, isOutput=False)
    sllb = nc.declare_dram_parameter("sllb", [P, S], f32, isOutput=False)
    emask = nc.declare_dram_parameter("emask", [nt, P, QC], bf16, isOutput=False)
    sel = nc.declare_dram_parameter("sel", [4, P, P], f32r, isOutput=False)
    out = nc.declare_dram_parameter("out", [S, D], bf16, isOutput=True)

    with TileContext(nc) as tc:
        with (
            tc.tile_pool(name="consts", bufs=1) as consts,
            tc.tile_pool(name="ps", bufs=2, space="PSUM") as ps_pool,
            tc.tile_pool(name="probs", bufs=4) as probs_pool,
            tc.tile_pool(name="oout", bufs=3) as oout_pool,
            tc.tile_pool(name="recip", bufs=3) as recip_pool,
        ):
          for _rep in range(reps):
            # ---- constant loads ----
            # weights first, then x in seq-quarter-major column slices so the
            # quarter-0 projections (everything attention chunk 0 needs) can
            # start after ~1/4 of the x bytes have landed
            aq_sb = consts.tile([P, 8, 256], bf16, tag="aq", name="aq")
            nc.sync.dma_start(out=aq_sb[:], in_=aq.rearrange("(ko ki) f -> ki ko f", ki=P))
            xk = []
            xT_r = xT.rearrange("(ko ki) f -> ki ko f", ki=P)
            for k in range(8):
                t = consts.tile([P, S], bf16, tag=f"xk{k}", name=f"xk{k}")
                xk.append(t)
            nc.sync.dma_start(out=xk[0][:], in_=xT_r[:, 0])
            ak_sb = consts.tile([P, 8, 256], bf16, tag="ak", name="ak")
            nc.sync.dma_start(out=ak_sb[:], in_=ak.rearrange("(ko ki) f -> ki ko f", ki=P))
            av_sb = consts.tile([P, 8, 256], bf16, tag="av", name="av")
            nc.sync.dma_start(out=av_sb[:], in_=av.rearrange("(ko ki) f -> ki ko f", ki=P))
            for k in range(1, 8):
                nc.sync.dma_start(out=xk[k][:], in_=xT_r[:, k])
            sll_sb = consts.tile([P, S], f32, tag="sll", name="sll")
            nc.sync.dma_start(out=sll_sb[:, :1024], in_=sllb[:, :1024])
            nc.sync.dma_start(out=sll_sb[:, 1024:], in_=sllb[:, 1024:])
            em_res = consts.tile([P, nt, QC], bf16, tag="emres", name="emres")
            nc.sync.dma_start(out=em_res[:], in_=emask.rearrange("t p f -> p t f"))
            sel_sb = consts.tile([P, 4, P], f32r, tag="sel", name="sel")
            nc.sync.dma_start(out=sel_sb[:], in_=sel.rearrange("t k m -> k t m"))
            wor_sb = consts.tile([P, 2, D], bf16, tag="wor", name="wor")
            nc.sync.dma_start(out=wor_sb[:], in_=wor.rearrange("(ko ki) f -> ki ko f", ki=P))

            qTh = [[consts.tile([P, 1024], f32r, tag=f"qT{g}{h}", name=f"qT{g}{h}") for h in range(2)] for g in range(2)]
            kTh = [[consts.tile([P, 1024], f32r, tag=f"kT{g}{h}", name=f"kT{g}{h}") for h in range(2)] for g in range(2)]
            # 64-wide per-head slot: cols 0:32 = v, col 32 = 1 (denominator
            # row), cols 33:64 = 0 so each PV matmul pair covers all 128 PSUM
            # partitions (no uninitialized rows, no per-chunk memset).
            vq = [consts.tile([P, 4, 8, 64], bf16, tag=f"vq{q}", name=f"vq{q}") for q in range(4)]
            attQ = [[consts.tile([P, QC], bf16, tag=f"att{g}{q}", name=f"att{g}{q}") for q in range(NQC)] for g in range(2)]

            def proj_qk_unit(w, g, half, c2):
                # one 512-wide chunk of the q or k projection for (g, half)
                lhs_sb = aq_sb if w == "q" else ak_sb
                base = 1024 * half + 512 * c2
                ps = ps_pool.tile([P, 512], f32, tag="pj", bufs=2, name="ps")
                for k in range(8):
                    nc.tensor.matmul(
                        ps[:],
                        lhsT=lhs_sb[:, k, 128 * g : 128 * g + 128],
                        rhs=xk[k][:, base : base + 512],
                        start=(k == 0),
                        stop=(k == 7),
                    )
                dst = (qTh if w == "q" else kTh)[g][half][:, 512 * c2 : 512 * c2 + 512]
                if w == "q":
                    nc.vector.tensor_tensor(dst, ps[:], sll_sb[:, base : base + 512], MULT)
                else:
                    nc.vector.tensor_copy(dst, ps[:])

            def qk_unit(w, g, half, c2):
                return [lambda: proj_qk_unit(w, g, half, c2)]

            def startup_unit():
                """Interleaved q/k (g0, half0, both c2) + v quarter 0, ordered
                k-major so the matmuls trail the xk chunk DMAs. Accumulators
                are spread over pj/sc/pv banks (one group per 2KB bank, since
                start=True marks the whole zero region)."""
                psq = ps_pool.tile([P, 512], f32, tag="pj", bufs=2, name="psq")
                psk = ps_pool.tile([P, 512], f32, tag="pj", bufs=2, name="psk")
                psv4 = ps_pool.tile([P, 1024], f32, tag="sc", bufs=2, name="psv4")
                psqk2 = ps_pool.tile([P, 1024], f32, tag="sc", bufs=2, name="psqk2")
                psv5 = ps_pool.tile([P, 1024], f32, tag="pv", bufs=1, name="psv5")
                vsl = [psv4[:, 0:256], psv4[:, 512:768], psv5[:, 0:256], psv5[:, 512:768]]
                nc.vector.memset(vq[0][:, :, :, 32:33], 1.0)
                nc.vector.memset(vq[0][:, :, :, 33:64], 0.0)
                for k in range(8):
                    nc.tensor.matmul(
                        psq[:], lhsT=aq_sb[:, k, 0:128], rhs=xk[k][:, 0:512],
                        start=(k == 0), stop=(k == 7),
                    )
                    nc.tensor.matmul(
                        psk[:], lhsT=ak_sb[:, k, 0:128], rhs=xk[k][:, 0:512],
                        start=(k == 0), stop=(k == 7),
                    )
                    nc.tensor.matmul(
                        psqk2[:, 0:512], lhsT=aq_sb[:, k, 0:128], rhs=xk[k][:, 512:1024],
                        start=(k == 0), stop=(k == 7), skip_group_check=True,
                    )
                    nc.tensor.matmul(
                        psqk2[:, 512:1024], lhsT=ak_sb[:, k, 0:128], rhs=xk[k][:, 512:1024],
                        start=(k == 0), stop=(k == 7), skip_group_check=True,
                    )
                    for st in range(4):
                        nc.tensor.matmul(
                            vsl[st],
                            lhsT=xk[k][:, 128 * st : 128 * st + 128],
                            rhs=av_sb[:, k, :],
                            start=(k == 0), stop=(k == 7),
                            skip_group_check=True,
                        )
                nc.vector.tensor_copy(kTh[0][0][:, 0:512], psk[:])
                nc.vector.tensor_tensor(qTh[0][0][:, 0:512], psq[:], sll_sb[:, 0:512], MULT)
                nc.vector.tensor_copy(kTh[0][0][:, 512:1024], psqk2[:, 512:1024])
                nc.vector.tensor_tensor(qTh[0][0][:, 512:1024], psqk2[:, 0:512], sll_sb[:, 512:1024], MULT)
                for st in range(4):
                    nc.vector.tensor_copy(
                        vq[0][:, st, :, 0:32],
                        vsl[st].rearrange("p (h c) -> p h c", h=8),
                    )

            def proj_v_unit(qq, sti):
                if sti == 0:
                    nc.vector.memset(vq[qq][:, :, :, 32:33], 1.0)
                    nc.vector.memset(vq[qq][:, :, :, 33:64], 0.0)
                st = 4 * qq + sti
                psv = ps_pool.tile([P, 512], f32, tag="pj", bufs=2, name="psv")
                for k in range(8):
                    nc.tensor.matmul(
                        psv[:, :256],
                        lhsT=xk[k][:, 128 * st : 128 * st + 128],
                        rhs=av_sb[:, k, :],
                        start=(k == 0),
                        stop=(k == 7),
                    )
                nc.vector.tensor_copy(
                    vq[qq][:, sti, :, 0:32],
                    psv[:, :256].rearrange("p (h c) -> p h c", h=8),
                )

            def v_quarter_units(qq):
                return [(lambda qq=qq, sti=sti: proj_v_unit(qq, sti)) for sti in range(4)]

            def wo_half(qc, sti, n, obs, tail=False):
                st = 4 * qc + sti
                if n == 0:
                    obs[sti] = oout_pool.tile([P, D], bf16, tag="ob", name="ob")
                ob = obs[sti]
                wops = ps_pool.tile([P, 512], f32, tag="pj", bufs=2, name="wops")
                for kk in range(2):
                    nc.tensor.matmul(
                        wops[:],
                        lhsT=attQ[kk][qc][:, 128 * sti : 128 * sti + 128],
                        rhs=wor_sb[:, kk, 512 * n : 512 * n + 512],
                        start=(kk == 0),
                        stop=(kk == 1),
                    )
                if tail and n == 0:
                    nc.scalar.copy(ob[:, :512], wops[:])
                else:
                    nc.vector.tensor_copy(ob[:, 512 * n : 512 * n + 512], wops[:])
                if n == 1:
                    nc.sync.dma_start(out=out[128 * st : 128 * st + 128, :], in_=ob[:])

            def wo_units(qc, tail=False):
                obs = {}
                return [
                    (lambda qc=qc, sti=sti, n=n, obs=obs: wo_half(qc, sti, n, obs, tail))
                    for sti in range(4)
                    for n in range(2)
                ]

            def compact_unit(g, qc, sabA, sabB):
                cpt = ps_pool.tile([P, 1024], f32, tag="sc", bufs=2, name="cpt")
                pvC = cpt[:, :512]
                dnC = cpt[:, 512:]
                # denominators first so the reciprocal overlaps the pv sels
                nc.tensor.matmul(dnC, lhsT=sel_sb[:, 2], rhs=sabA[:], start=True, stop=False, skip_group_check=True)
                nc.tensor.matmul(dnC, lhsT=sel_sb[:, 3], rhs=sabB[:], start=False, stop=True, skip_group_check=True)
                nc.tensor.matmul(pvC, lhsT=sel_sb[:, 0], rhs=sabA[:], start=True, stop=False, skip_group_check=True)
                nc.tensor.matmul(pvC, lhsT=sel_sb[:, 1], rhs=sabB[:], start=False, stop=True, skip_group_check=True)
                rc = recip_pool.tile([P, QC], f32, tag="rc", name="rc")
                nc.vector.reciprocal_approx_fast(out=rc[:], in_=dnC)
                nc.vector.tensor_tensor(attQ[g][qc][:], pvC, rc[:], MULT)

            def compact_half(g, qc, sabA, sabB, hb, use_pj=False):
                # tail variant: one 256-column half per fresh tile pair (one
                # accumulation group per PSUM bank), so the first wo matmuls
                # start as soon as the first half of attQ is ready. use_pj
                # avoids the sc-ring WAR on the last entry's exp.
                lo = 256 * hb
                if use_pj:
                    pvh = ps_pool.tile([P, 512], f32, tag="pj", bufs=2, name="cpv")[:, 0:256]
                    dnh = ps_pool.tile([P, 512], f32, tag="pj", bufs=2, name="cdn")[:, 0:256]
                else:
                    cpt = ps_pool.tile([P, 1024], f32, tag="sc", bufs=2, name="cpth")
                    pvh = cpt[:, 0:256]
                    dnh = cpt[:, 512:768]
                nc.tensor.matmul(dnh, lhsT=sel_sb[:, 2], rhs=sabA[:, lo : lo + 256], start=True, stop=False, skip_group_check=True)
                nc.tensor.matmul(dnh, lhsT=sel_sb[:, 3], rhs=sabB[:, lo : lo + 256], start=False, stop=True, skip_group_check=True)
                nc.tensor.matmul(pvh, lhsT=sel_sb[:, 0], rhs=sabA[:, lo : lo + 256], start=True, stop=False, skip_group_check=True)
                nc.tensor.matmul(pvh, lhsT=sel_sb[:, 1], rhs=sabB[:, lo : lo + 256], start=False, stop=True, skip_group_check=True)
                rc = recip_pool.tile([P, 256], f32, tag="rc", name="rc")
                nc.vector.reciprocal_approx_fast(out=rc[:], in_=dnh)
                nc.vector.tensor_tensor(attQ[g][qc][:, lo : lo + 256], pvh, rc[:], MULT)

            def attention_g(qc, g, fillers, pending, post_fill=(), ration=2):
                """Emit the entry stream for (qc, g); returns the deferred
                compaction closure (emitted by the caller into the next
                group's stream). `pending` closures are flushed after the
                first entry's scores; `post_fill` units join the filler deque
                right after that flush (they depend on the pending work)."""
                entries = plans[qc]
                qh, qcol = qc // 2, 512 * (qc % 2)
                if not entries:
                    nc.vector.memset(attQ[g][qc][:], 0.0)
                    while pending:
                        pending.popleft()()
                    for u in post_fill:
                        u()
                    return None
                pvdn = ps_pool.tile([P, 1024], f32, tag="pv", bufs=1, name="pvdn")
                nent = len(entries)
                prev = None  # (pr tile, kt, first, zq)
                tail = qc == NQC - 1 and g == 1
                # index of the last entry touching columns < 256 (resp < 384):
                # once its PV is emitted, those pv/dn columns are final and
                # can be copied out while the remaining entries run
                h0_ei = max(
                    (i for i, e in enumerate(entries) if e[2] < 256), default=-1
                )
                q2_ei = max(
                    (i for i, e in enumerate(entries) if e[2] < 384), default=-1
                )
                sabA = recip_pool.tile([P, QC], f32r, tag="sabA", name="sabA")
                sabB = recip_pool.tile([P, QC], f32r, tag="sabB", name="sabB")
                ec = [0]  # columns [0, ec) already copied out

                def emit_sab_cols(hi):
                    lo = ec[0]
                    if hi <= lo:
                        return
                    nc.vector.tensor_copy(sabA[:, lo:hi], pvdn[:, lo:hi])
                    nc.scalar.copy(sabB[:, lo:hi], pvdn[:, 512 + lo : 512 + hi])
                    ec[0] = hi

                def emit_pv(pv_args, stop=False):
                    pr, kt, first, zq = pv_args
                    for j in range(4):
                        bank = pvdn[:, :512] if j < 2 else pvdn[:, 512:]
                        idx = j % 2
                        nc.tensor.matmul(
                            bank[64 * idx : 64 * idx + 64, zq:],
                            lhsT=vq[kt // 4][:, kt % 4, 4 * g + j, :],
                            rhs=pr[:, 512 * j + zq : 512 * j + 512],
                            start=first,
                            stop=(stop and j == 3),
                            tile_position=(0, 64 * idx),
                            skip_group_check=True,
                        )

                for ei, (kt, mi, zq, w1) in enumerate(entries):
                    if fillers and (
                        ei % ration == 1 or (ration > 2 and ei >= nent - 4)
                    ):
                        pop_filler()
                    require(("k", g, kt // 8, (kt % 8) // 4))
                    require(("v", kt // 4))
                    kh, kcol = kt // 8, 128 * (kt % 8)
                    psa = ps_pool.tile([P, 1024], f32, tag="sc", bufs=2, name="psa")
                    psb = ps_pool.tile([P, 1024], f32, tag="sc", bufs=2, name="psb")
                    for j in range(4):
                        dst = (psa if j < 2 else psb)[:, 512 * (j % 2) + zq : 512 * (j % 2) + 512]
                        nc.tensor.matmul(
                            dst,
                            lhsT=kTh[g][kh][32 * j : 32 * j + 32, kcol : kcol + 128],
                            rhs=qTh[g][qh][32 * j : 32 * j + 32, qcol + zq : qcol + 512],
                            start=True,
                            stop=True,
                            tile_position=(32 * j, 0),
                        )
                    pr = probs_pool.tile([P, 2048], bf16, tag="pr", name="pr")
                    if zq == 0:
                        nc.scalar.activation(pr[:, :1024], psa[:], EXP)
                        nc.scalar.activation(pr[:, 1024:], psb[:], EXP)
                    else:
                        pra = pr[:, :1024].rearrange("p (t f) -> p t f", t=2)[:, :, zq:]
                        prb = pr[:, 1024:].rearrange("p (t f) -> p t f", t=2)[:, :, zq:]
                        psa3 = psa[:].rearrange("p (t f) -> p t f", t=2)[:, :, zq:]
                        psb3 = psb[:].rearrange("p (t f) -> p t f", t=2)[:, :, zq:]
                        nc.scalar.activation(pra, psa3, EXP)
                        nc.scalar.activation(prb, psb3, EXP)
                    if ei == min(1, nent - 1):
                        while pending:
                            pending.popleft()()
                        fillers.extend(post_fill)
                    if ei + 1 < nent:
                        nkt = entries[ei + 1][0]
                        require(("k", g, nkt // 8, (nkt % 8) // 4))
                        require(("v", nkt // 4))
                    if mi >= 0 and w1 > zq:
                        w = w1 - zq
                        pr3 = pr[:].rearrange("p (h f) -> p h f", h=4)[:, :, zq:w1]
                        emt = em_res[:, mi, :]
                        nc.vector.tensor_tensor(
                            pr3, pr3, emt[:, None, zq:w1].to_broadcast((P, 4, w)), MULT
                        )
                    # PV trails by one entry: by the time PV(e-1) issues, its
                    # exps are done and the next scores' PSUM WAR has aged out
                    if prev is not None:
                        emit_pv(prev)
                        if tail:
                            if ei - 1 == h0_ei:
                                emit_sab_cols(256)
                            if ei - 1 == q2_ei:
                                emit_sab_cols(384)
                    prev = (pr, kt, ei == 0, zq)
                emit_pv(prev, stop=True)
                # eager pv->sbuf copy so the pv accumulator frees right away
                if tail:
                    emit_sab_cols(512)
                else:
                    nc.vector.tensor_copy(sabA[:], pvdn[:, :512])
                    nc.vector.tensor_copy(sabB[:], pvdn[:, 512:])
                return (g, qc, sabA, sabB)

            # ---- resource-tracked emission ----
            class Unit:
                __slots__ = ("fn", "done")

                def __init__(self, fn):
                    self.fn = fn
                    self.done = False

                def __call__(self):
                    if not self.done:
                        self.done = True
                        self.fn()

            done: set = set()
            queued: dict = {}
            fillers: deque = deque()
            pending: deque = deque()

            def resource_units(r):
                if r[0] == "v":
                    return v_quarter_units(r[1])
                w, g, half, c2 = r
                return qk_unit(w, g, half, c2)

            def needs(qc, g):
                res = []
                for kt, _, _, _ in plans[qc]:
                    res.append(("k", g, kt // 8, (kt % 8) // 4))
                res.append(("q", g, qc // 2, qc % 2))
                for kt, _, _, _ in plans[qc]:
                    res.append(("v", kt // 4))
                seen = []
                for r in res:
                    if r not in seen:
                        seen.append(r)
                return seen

            def require(r):
                if r in done:
                    return
                done.add(r)
                for u in queued.pop(r, None) or [Unit(f) for f in resource_units(r)]:
                    u()

            def queue(r):
                if r in done or r in queued:
                    return
                units = [Unit(f) for f in resource_units(r)]
                queued[r] = units
                fillers.extend(units)

            def pop_filler():
                while fillers:
                    u = fillers.popleft()
                    if not u.done:
                        u()
                        return

            startup_unit()
            done.update(
                {("q", 0, 0, 0), ("k", 0, 0, 0), ("q", 0, 0, 1), ("k", 0, 0, 1), ("v", 0)}
            )
            tail_args = None

            qc_order = list(range(NQC))
            # wo fillers deferred toward the later (larger) chunks, which have
            # the most attention entries to fill and the least projection work
            if NQC == 4:
                wo_sched = {(2, 0): 0, (3, 0): 1, (3, 1): 2}
            else:
                wo_sched = {(q, 0): q - 1 for q in range(1, NQC)}
            for qi, qc in enumerate(qc_order):
                for g in range(2):
                    require(("q", g, qc // 2, qc % 2))
                    # prefetch upcoming resources into the filler stream
                    post_fill = ()
                    if (qc, g) in wo_sched:
                        post_fill = [Unit(f) for f in wo_units(wo_sched[(qc, g)])]
                    if g == 0:
                        for r in needs(qc, 1):
                            queue(r)
                    elif qi + 1 < NQC:
                        for r in needs(qc_order[qi + 1], 0):
                            queue(r)
                        for r in needs(qc_order[qi + 1], 1):
                            queue(r)
                    # slower filler cadence in the final group: leftover units
                    # then flush right before the tail compaction, filling its
                    # otherwise-stalled dependency chain
                    last = qi == NQC - 1 and g == 1
                    cu = attention_g(qc, g, fillers, pending, post_fill,
                                     ration=3 if last else 2)
                    if cu is not None:
                        if last:
                            tail_args = cu
                        else:
                            pending.append(lambda a=cu: compact_unit(*a))
            while pending:
                pending.popleft()()
            while fillers:
                fillers.popleft()()
            if tail_args is not None:
                # wo(last qc) with kk=0 matmuls pre-started (attQ[0] is ready
                # well before the final compaction finishes attQ[1])
                lq = qc_order[-1]
                obs: dict = {}
                pres: dict = {}

                def wo_pre(sti):
                    obs[sti] = oout_pool.tile([P, D], bf16, tag="ob", name="ob")
                    ws = []
                    for n in range(2):
                        w = ps_pool.tile([P, 512], f32, tag="pj", bufs=2, name="wopt")
                        nc.tensor.matmul(
                            w[:],
                            lhsT=attQ[0][lq][:, 128 * sti : 128 * sti + 128],
                            rhs=wor_sb[:, 0, 512 * n : 512 * n + 512],
                            start=True, stop=False,
                        )
                        ws.append(w)
                    pres[sti] = ws

                def wo_fin(sti):
                    ob = obs[sti]
                    rows = slice(128 * (4 * lq + sti), 128 * (4 * lq + sti) + 128)
                    for n in range(2):
                        nc.tensor.matmul(
                            pres[sti][n][:],
                            lhsT=attQ[1][lq][:, 128 * sti : 128 * sti + 128],
                            rhs=wor_sb[:, 1, 512 * n : 512 * n + 512],
                            start=False, stop=True,
                        )
                        if n == 0:
                            nc.scalar.copy(ob[:, :512], pres[sti][n][:])
                        else:
                            nc.vector.tensor_copy(ob[:, 512:], pres[sti][n][:])
                    nc.sync.dma_start(out=out[rows, :], in_=ob[:])

                wo_pre(0)
                compact_half(*tail_args, 0)
                wo_fin(0)
                wo_pre(1)
                wo_fin(1)
                compact_half(*tail_args, 1)
                wo_pre(2)
                wo_fin(2)
                wo_pre(3)
                wo_fin(3)
            else:
                for u in wo_units(qc_order[-1], tail=True):
                    u()

    if not nc.is_finalized():
        nc.finalize()
    return nc


def _round_f32r(a):
    """Round fp32 array to the PE's f32r format (mantissa truncated to 11
    bits, round-to-nearest-even at bit 12)."""
    u = np.ascontiguousarray(a, dtype=np.float32).view(np.uint32)
    u2 = (u + np.uint32(0x7FF) + ((u >> np.uint32(12)) & np.uint32(1))) & np.uint32(0xFFFFF000)
    return u2.view(np.float32)


def _plan_from_mask(mask):
    """Classify [128, 512] tiles of exp(mask)^T; returns (plans, packed_tiles).

    Per tile: zq = leading fully-masked q-columns (even), w1 = end of the
    column window that still contains any masked element; columns >= w1 are
    all-ones and skip the multiply."""
    em = np.exp(mask.astype(np.float32))  # [q, k]
    emT = np.ascontiguousarray(em.T)  # [k, q]
    plans = []
    tiles = []
    tile_keys = {}
    for qc in range(NQC):
        ent = []
        for kt in range(NKT):
            t = emT[P * kt : P * (kt + 1), QC * qc : QC * (qc + 1)]
            if not t.any():
                continue  # fully masked out: skip tile entirely
            if (t == 1.0).all():
                ent.append((kt, -1, 0, 0))
                continue
            nz = np.flatnonzero(t.any(axis=0))
            zq = (int(nz[0]) // 2 * 2) if len(nz) else 0
            not_ones = np.flatnonzero(~(t == 1.0).all(axis=0))
            w1 = int(not_ones[-1]) + 1 if len(not_ones) else 0
            if w1 <= zq:
                ent.append((kt, -1, zq, 0))
                continue
            key = t.tobytes()
            mi = tile_keys.get(key)
            if mi is None:
                mi = len(tiles)
                tile_keys[key] = mi
                tiles.append(t.astype(ml_dtypes.bfloat16))
            ent.append((kt, mi, zq, w1))
        if ent and min(z for _, _, z, _ in ent) > 0:
            # every tile skips some leading columns -> those pv/dn columns
            # would never be written; disable skipping for this chunk
            ent = [(kt, mi, 0, w1) for kt, mi, _, w1 in ent]
        plans.append(tuple(ent))
    if tiles:
        packed = np.ascontiguousarray(np.stack(tiles))
    else:
        packed = np.zeros((1, P, QC), dtype=ml_dtypes.bfloat16)
    return tuple(plans), packed


def _sel_mats():
    s = np.zeros((4, P, P), dtype=np.float32)
    for m in range(32):
        s[0, m, m] = 1.0            # pvC rows 0-31   <- bankA rows 0-31
        s[0, m + 64, m + 32] = 1.0  # pvC rows 32-63  <- bankA rows 64-95
        s[1, m, m + 64] = 1.0       # pvC rows 64-95  <- bankB rows 0-31
        s[1, m + 64, m + 96] = 1.0  # pvC rows 96-127 <- bankB rows 64-95
    s[2, 32, 0:32] = 1.0            # dnC rows 0-31   <- bankA row 32
    s[2, 96, 32:64] = 1.0           # dnC rows 32-63  <- bankA row 96
    s[3, 32, 64:96] = 1.0           # dnC rows 64-95  <- bankB row 32
    s[3, 96, 96:128] = 1.0          # dnC rows 96-127 <- bankB row 96
    return s


def kernel(x, mask, section_log_len, wq, wk, wv, wo, seq_scale):
    from concourse.bass_utils import run_bass_kernel_spmd

    x = np.asarray(x, dtype=np.float32)
    assert x.shape == (B, S, D), x.shape
    mask2 = np.asarray(mask, dtype=np.float32).reshape(S, S)
    sll = np.asarray(section_log_len, dtype=np.float32).reshape(S)
    ss = np.asarray(seq_scale, dtype=np.float32).reshape(H)
    wq = np.asarray(wq, dtype=np.float32)
    wk = np.asarray(wk, dtype=np.float32)
    wv = np.asarray(wv, dtype=np.float32)
    wo = np.asarray(wo, dtype=np.float32)

    plans, tiles = _plan_from_mask(mask2)
    key = (plans, tiles.shape[0])
    nc = _GRAPH_CACHE.get(key)
    if nc is None:
        nc = _build_graph(plans, tiles.shape[0])
        _GRAPH_CACHE[key] = nc

    bf = ml_dtypes.bfloat16
    xT = [np.ascontiguousarray(x[b].T).astype(bf) for b in range(B)]
    selm = _sel_mats()
    sllB = np.ascontiguousarray(
        np.broadcast_to(sll[None, :], (P, S)), dtype=np.float32
    )

    in_maps = []
    for c in range(NCORES):
        b, g2 = divmod(c, 4)
        rows = slice(256 * g2, 256 * (g2 + 1))
        ssr = np.repeat(ss[8 * g2 : 8 * g2 + 8], HD) / math.sqrt(HD)
        in_maps.append(
            {
                "xT": xT[b],
                "aq": np.ascontiguousarray((wq[rows, :] * ssr[:, None]).T).astype(bf),
                "ak": np.ascontiguousarray(wk[rows, :].T).astype(bf),
                "av": np.ascontiguousarray(wv[rows, :].T).astype(bf),
                "wor": np.ascontiguousarray(wo[:, rows].T).astype(bf),
                "sllb": sllB,
                "emask": tiles,
                "sel": _round_f32r(selm),
            }
        )

    res = run_bass_kernel_spmd(nc, in_maps, core_ids=list(range(NCORES))).results
    out = np.zeros((B, S, D), dtype=np.float32)
    for c in range(NCORES):
        out[c // 4] += np.asarray(res[c]["out"], dtype=np.float32)
    return out


# revision 83
# speedup vs baseline: 1.0034x; 1.0034x over previous
"""Trainium2 Bass kernel for nn_Attention_80917183857290.

Multi-head causal attention (B=2, S=2048, D=1024, H=32, HD=32) with
SSMax-style per-query log-length score scaling, run SPMD on 8 NeuronCores.

Sharding: core c -> batch b = c // 4, head-group g2 = c % 4 (8 heads each).
Per core:
  - projections q,k (transposed layout [head_dim rows, seq]) and v, from
    bf16 x^T / bf16 weights (psum accumulates fp32; q/k kept f32r on-chip);
    a fused startup unit interleaves the first q/k/v matmuls k-major so they
    trail the x chunk DMAs
  - scores^T[k, q] per head via row-tiled K=32 matmuls (4 heads packed into
    the PE array via tile_position; two 2-bank PSUM tiles per k-tile entry,
    double-buffered)
  - probs = exp(scores) * emask (bf16), with the emask multiply narrowed to
    the partial-mask column window; fully-masked leading columns skipped (zq)
  - out^T[hd, q] and softmax denominators in one pass: V is ones-augmented
    and zero-padded to a 64-wide slot ([v_h | 1 | 0...]) so the two PV
    matmuls per bank cover all 128 partitions; PV accumulates into a
    per-(group, chunk) 2-bank PSUM tile with start=True on the first entry
    (no memset). PV matmuls trail the entry stream by one entry so the
    next entry's scores never wait on exp/PSUM WAR hazards.
  - compaction (deferred into the next group's entry stream): DVE copies
    pv/dn to SBUF, host-built permutation matmuls compact 4 heads +
    replicated denominators into a fresh 2-bank tile; att = pv * recip(dn)
    -> bf16; partial = att^T.T @ wo via PE. The final chunk's compaction is
    split into column halves interleaved with the last wo units, whose
    kk=0 matmuls are pre-started.
  - host sums the 4 partial outputs per batch.

PSUM budget (8 banks): pv tag [P,1024]x1 + sc tag [P,1024]x2 (scores double
buffer, also reused for the compaction output) + pj tag [P,512]x2
(projection / wo accumulators) = 16KB per partition exactly.

Emission is resource-tracked: projection units are required lazily
just-in-time per entry and otherwise queued as one-shot filler units popped
between attention entries (wo units split in half for finer granularity),
so the PE never starves while ACT runs exp; wo filler units are deferred
toward the later (larger) attention chunks. In the final group the pv/dn
SBUF copies are split across DVE and ACT and the first-half columns are
copied out two entries early (the remaining causal entries only touch
columns >= 256), shortening the tail chain. TimelineSim: 203023 ns
(baseline 258710 ns).
"""

import math
from collections import deque

import numpy as np
import ml_dtypes

B, S, D, H = 2, 2048, 1024, 32
HD = D // H  # 32
P = 128
QC = 512  # q-chunk (PSUM bank free size, fp32)
NQC = S // QC  # 4
NKT = S // P  # 16
NCORES = 8

_GRAPH_CACHE: dict = {}


def _build_graph(plans, nt, reps=1):
    """Build the per-core Bass graph.

    plans: tuple over qc (4) of tuple of (kt, mask_idx, zq, w1) entries;
      mask_idx -1 means no mask multiply; >=0 indexes the packed emask tile
      array, multiplied over columns [zq, w1).
    nt: number of packed [128, 512] bf16 exp-mask tiles (>= 1).
    """
    import concourse.mybir as mybir
    from concourse import bacc
    from concourse.tile import TileContext

    f32 = mybir.dt.float32
    f32r = mybir.dt.float32r
    bf16 = mybir.dt.bfloat16
    EXP = mybir.ActivationFunctionType.Exp
    MULT = mybir.AluOpType.mult

    nc = bacc.Bacc()

    xT = nc.declare_dram_parameter("xT", [D, S], bf16, isOutput=False)
    aq = nc.declare_dram_parameter("aq", [D, 256], bf16, isOutput=False)
    ak = nc.declare_dram_parameter("ak", [D, 256], bf16, isOutput=False)
    av = nc.declare_dram_parameter("av", [D, 256], bf16, isOutput=False)
    wor = nc.declare_dram_parameter("wor", [256, D], bf16, isOutput=False)
    sllb = nc.declare_dram_parameter("sllb", [P, S], f32, isOutput=False)
    emask = nc.declare_dram_parameter("emask", [nt, P, QC], bf16, isOutput=False)
    sel = nc.declare_dram_parameter("sel", [4, P, P], f32r, isOutput=False)
    out = nc.declare_dram_parameter("out", [S, D], bf16, isOutput=True)

    with TileContext(nc) as tc:
        with (
            tc.tile_pool(name="consts", bufs=1) as consts,
            tc.tile_pool(name="ps", bufs=2, space="PSUM") as ps_pool,
            tc.tile_pool(name="probs", bufs=4) as probs_pool,
            tc.tile_pool(name="oout", bufs=3) as oout_pool,
            tc.tile_pool(name="recip", bufs=3) as recip_pool,
        ):
          for _rep in range(reps):
            # ---- constant loads ----
            # weights first, then x in seq-quarter-major column slices so the
            # quarter-0 projections (everything attention chunk 0 needs) can
            # start after ~1/4 of the x bytes have landed
            aq_sb = consts.tile([P, 8, 256], bf16, tag="aq", name="aq")
            nc.sync.dma_start(out=aq_sb[:], in_=aq.rearrange("(ko ki) f -> ki ko f", ki=P))
            xk = []
            xT_r = xT.rearrange("(ko ki) f -> ki ko f", ki=P)
            for k in range(8):
                t = consts.tile([P, S], bf16, tag=f"xk{k}", name=f"xk{k}")
                xk.append(t)
            nc.sync.dma_start(out=xk[0][:], in_=xT_r[:, 0])
            ak_sb = consts.tile([P, 8, 256], bf16, tag="ak", name="ak")
            nc.sync.dma_start(out=ak_sb[:], in_=ak.rearrange("(ko ki) f -> ki ko f", ki=P))
            av_sb = consts.tile([P, 8, 256], bf16, tag="av", name="av")
            nc.sync.dma_start(out=av_sb[:], in_=av.rearrange("(ko ki) f -> ki ko f", ki=P))
            for k in range(1, 8):
                nc.sync.dma_start(out=xk[k][:], in_=xT_r[:, k])
            sll_sb = consts.tile([P, S], f32, tag="sll", name="sll")
            nc.sync.dma_start(out=sll_sb[:, :1024], in_=sllb[:, :1024])
            nc.sync.dma_start(out=sll_sb[:, 1024:], in_=sllb[:, 1024:])
            em_res = consts.tile([P, nt, QC], bf16, tag="emres", name="emres")
            nc.sync.dma_start(out=em_res[:], in_=emask.rearrange("t p f -> p t f"))
            sel_sb = consts.tile([P, 4, P], f32r, tag="sel", name="sel")
            nc.sync.dma_start(out=sel_sb[:], in_=sel.rearrange("t k m -> k t m"))
            wor_sb = consts.tile([P, 2, D], bf16, tag="wor", name="wor")
            nc.sync.dma_start(out=wor_sb[:], in_=wor.rearrange("(ko ki) f -> ki ko f", ki=P))

            qTh = [[consts.tile([P, 1024], f32r, tag=f"qT{g}{h}", name=f"qT{g}{h}") for h in range(2)] for g in range(2)]
            kTh = [[consts.tile([P, 1024], f32r, tag=f"kT{g}{h}", name=f"kT{g}{h}") for h in range(2)] for g in range(2)]
            # 64-wide per-head slot: cols 0:32 = v, col 32 = 1 (denominator
            # row), cols 33:64 = 0 so each PV matmul pair covers all 128 PSUM
            # partitions (no uninitialized rows, no per-chunk memset).
            vq = [consts.tile([P, 4, 8, 64], bf16, tag=f"vq{q}", name=f"vq{q}") for q in range(4)]
            attQ = [[consts.tile([P, QC], bf16, tag=f"att{g}{q}", name=f"att{g}{q}") for q in range(NQC)] for g in range(2)]

            def proj_qk_unit(w, g, half, c2):
                # one 512-wide chunk of the q or k projection for (g, half)
                lhs_sb = aq_sb if w == "q" else ak_sb
                base = 1024 * half + 512 * c2
                ps = ps_pool.tile([P, 512], f32, tag="pj", bufs=2, name="ps")
                for k in range(8):
                    nc.tensor.matmul(
                        ps[:],
                        lhsT=lhs_sb[:, k, 128 * g : 128 * g + 128],
                        rhs=xk[k][:, base : base + 512],
                        start=(k == 0),
                        stop=(k == 7),
                    )
                dst = (qTh if w == "q" else kTh)[g][half][:, 512 * c2 : 512 * c2 + 512]
                if w == "q":
                    nc.vector.tensor_tensor(dst, ps[:], sll_sb[:, base : base + 512], MULT)
                else:
                    nc.vector.tensor_copy(dst, ps[:])

            def qk_unit(w, g, half, c2):
                return [lambda: proj_qk_unit(w, g, half, c2)]

            def startup_unit():
                """Interleaved q/k (g0, half0, both c2) + v quarter 0, ordered
                k-major so the matmuls trail the xk chunk DMAs. Accumulators
                are spread over pj/sc/pv banks (one group per 2KB bank, since
                start=True marks the whole zero region)."""
                psq = ps_pool.tile([P, 512], f32, tag="pj", bufs=2, name="psq")
                psk = ps_pool.tile([P, 512], f32, tag="pj", bufs=2, name="psk")
                psv4 = ps_pool.tile([P, 1024], f32, tag="sc", bufs=2, name="psv4")
                psqk2 = ps_pool.tile([P, 1024], f32, tag="sc", bufs=2, name="psqk2")
                psv5 = ps_pool.tile([P, 1024], f32, tag="pv", bufs=1, name="psv5")
                vsl = [psv4[:, 0:256], psv4[:, 512:768], psv5[:, 0:256], psv5[:, 512:768]]
                nc.vector.memset(vq[0][:, :, :, 32:33], 1.0)
                nc.vector.memset(vq[0][:, :, :, 33:64], 0.0)
                for k in range(8):
                    nc.tensor.matmul(
                        psq[:], lhsT=aq_sb[:, k, 0:128], rhs=xk[k][:, 0:512],
                        start=(k == 0), stop=(k == 7),
                    )
                    nc.tensor.matmul(
                        psk[:], lhsT=ak_sb[:, k, 0:128], rhs=xk[k][:, 0:512],
                        start=(k == 0), stop=(k == 7),
                    )
                    nc.tensor.matmul(
                        psqk2[:, 0:512], lhsT=aq_sb[:, k, 0:128], rhs=xk[k][:, 512:1024],
                        start=(k == 0), stop=(k == 7), skip_group_check=True,
                    )
                    nc.tensor.matmul(
                        psqk2[:, 512:1024], lhsT=ak_sb[:, k, 0:128], rhs=xk[k][:, 512:1024],
                        start=(k == 0), stop=(k == 7), skip_group_check=True,
                    )
                    for st in range(4):
                        nc.tensor.matmul(
                            vsl[st],
                            lhsT=xk[k][:, 128 * st : 128 * st + 128],
                            rhs=av_sb[:, k, :],
                            start=(k == 0), stop=(k == 7),
                            skip_group_check=True,
                        )
                nc.vector.tensor_copy(kTh[0][0][:, 0:512], psk[:])
                nc.vector.tensor_tensor(qTh[0][0][:, 0:512], psq[:], sll_sb[:, 0:512], MULT)
                nc.vector.tensor_copy(kTh[0][0][:, 512:1024], psqk2[:, 512:1024])
                nc.vector.tensor_tensor(qTh[0][0][:, 512:1024], psqk2[:, 0:512], sll_sb[:, 512:1024], MULT)
                for st in range(4):
                    nc.vector.tensor_copy(
                        vq[0][:, st, :, 0:32],
                        vsl[st].rearrange("p (h c) -> p h c", h=8),
                    )

            def proj_v_unit(qq, sti):
                if sti == 0:
                    nc.vector.memset(vq[qq][:, :, :, 32:33], 1.0)
                    nc.vector.memset(vq[qq][:, :, :, 33:64], 0.0)
                st = 4 * qq + sti
                psv = ps_pool.tile([P, 512], f32, tag="pj", bufs=2, name="psv")
                for k in range(8):
                    nc.tensor.matmul(
                        psv[:, :256],
                        lhsT=xk[k][:, 128 * st : 128 * st + 128],
                        rhs=av_sb[:, k, :],
                        start=(k == 0),
                        stop=(k == 7),
                    )
                nc.vector.tensor_copy(
                    vq[qq][:, sti, :, 0:32],
                    psv[:, :256].rearrange("p (h c) -> p h c", h=8),
                )

            def v_quarter_units(qq):
                return [(lambda qq=qq, sti=sti: proj_v_unit(qq, sti)) for sti in range(4)]

            def wo_half(qc, sti, n, obs, tail=False):
                st = 4 * qc + sti
                if n == 0:
                    obs[sti] = oout_pool.tile([P, D], bf16, tag="ob", name="ob")
                ob = obs[sti]
                wops = ps_pool.tile([P, 512], f32, tag="pj", bufs=2, name="wops")
                for kk in range(2):
                    nc.tensor.matmul(
                        wops[:],
                        lhsT=attQ[kk][qc][:, 128 * sti : 128 * sti + 128],
                        rhs=wor_sb[:, kk, 512 * n : 512 * n + 512],
                        start=(kk == 0),
                        stop=(kk == 1),
                    )
                if tail and n == 0:
                    nc.scalar.copy(ob[:, :512], wops[:])
                else:
                    nc.vector.tensor_copy(ob[:, 512 * n : 512 * n + 512], wops[:])
                if n == 1:
                    nc.sync.dma_start(out=out[128 * st : 128 * st + 128, :], in_=ob[:])

            def wo_units(qc, tail=False):
                obs = {}
                return [
                    (lambda qc=qc, sti=sti, n=n, obs=obs: wo_half(qc, sti, n, obs, tail))
                    for sti in range(4)
                    for n in range(2)
                ]

            def compact_unit(g, qc, sabA, sabB):
                cpt = ps_pool.tile([P, 1024], f32, tag="sc", bufs=2, name="cpt")
                pvC = cpt[:, :512]
                dnC = cpt[:, 512:]
                # denominators first so the reciprocal overlaps the pv sels
                nc.tensor.matmul(dnC, lhsT=sel_sb[:, 2], rhs=sabA[:], start=True, stop=False, skip_group_check=True)
                nc.tensor.matmul(dnC, lhsT=sel_sb[:, 3], rhs=sabB[:], start=False, stop=True, skip_group_check=True)
                nc.tensor.matmul(pvC, lhsT=sel_sb[:, 0], rhs=sabA[:], start=True, stop=False, skip_group_check=True)
                nc.tensor.matmul(pvC, lhsT=sel_sb[:, 1], rhs=sabB[:], start=False, stop=True, skip_group_check=True)
                rc = recip_pool.tile([P, QC], f32, tag="rc", name="rc")
                nc.vector.reciprocal_approx_fast(out=rc[:], in_=dnC)
                nc.vector.tensor_tensor(attQ[g][qc][:], pvC, rc[:], MULT)

            def compact_half(g, qc, sabA, sabB, hb, use_pj=False):
                # tail variant: one 256-column half per fresh tile pair (one
                # accumulation group per PSUM bank), so the first wo matmuls
                # start as soon as the first half of attQ is ready. use_pj
                # avoids the sc-ring WAR on the last entry's exp.
                lo = 256 * hb
                if use_pj:
                    pvh = ps_pool.tile([P, 512], f32, tag="pj", bufs=2, name="cpv")[:, 0:256]
                    dnh = ps_pool.tile([P, 512], f32, tag="pj", bufs=2, name="cdn")[:, 0:256]
                else:
                    cpt = ps_pool.tile([P, 1024], f32, tag="sc", bufs=2, name="cpth")
                    pvh = cpt[:, 0:256]
                    dnh = cpt[:, 512:768]
                nc.tensor.matmul(dnh, lhsT=sel_sb[:, 2], rhs=sabA[:, lo : lo + 256], start=True, stop=False, skip_group_check=True)
                nc.tensor.matmul(dnh, lhsT=sel_sb[:, 3], rhs=sabB[:, lo : lo + 256], start=False, stop=True, skip_group_check=True)
                nc.tensor.matmul(pvh, lhsT=sel_sb[:, 0], rhs=sabA[:, lo : lo + 256], start=True, stop=False, skip_group_check=True)
                nc.tensor.matmul(pvh, lhsT=sel_sb[:, 1], rhs=sabB[:, lo : lo + 256], start=False, stop=True, skip_group_check=True)
                rc = recip_pool.tile([P, 256], f32, tag="rc", name="rc")
                nc.vector.reciprocal_approx_fast(out=rc[:], in_=dnh)
                nc.vector.tensor_tensor(attQ[g][qc][:, lo : lo + 256], pvh, rc[:], MULT)

            def attention_g(qc, g, fillers, pending, post_fill=(), ration=2):
                """Emit the entry stream for (qc, g); returns the deferred
                compaction closure (emitted by the caller into the next
                group's stream). `pending` closures are flushed after the
                first entry's scores; `post_fill` units join the filler deque
                right after that flush (they depend on the pending work)."""
                entries = plans[qc]
                qh, qcol = qc // 2, 512 * (qc % 2)
                if not entries:
                    nc.vector.memset(attQ[g][qc][:], 0.0)
                    while pending:
                        pending.popleft()()
                    for u in post_fill:
                        u()
                    return None
                pvdn = ps_pool.tile([P, 1024], f32, tag="pv", bufs=1, name="pvdn")
                nent = len(entries)
                prev = None  # (pr tile, kt, first, zq)
                tail = qc == NQC - 1 and g == 1
                # index of the last entry touching columns < 256: once its PV
                # is emitted, the first-half pv/dn columns are final
                h0_ei = max(
                    (i for i, e in enumerate(entries) if e[2] < 256), default=-1
                )
                sabA = recip_pool.tile([P, QC], f32r, tag="sabA", name="sabA")
                sabB = recip_pool.tile([P, QC], f32r, tag="sabB", name="sabB")

                def emit_pv(pv_args, stop=False):
                    pr, kt, first, zq = pv_args
                    for j in range(4):
                        bank = pvdn[:, :512] if j < 2 else pvdn[:, 512:]
                        idx = j % 2
                        nc.tensor.matmul(
                            bank[64 * idx : 64 * idx + 64, zq:],
                            lhsT=vq[kt // 4][:, kt % 4, 4 * g + j, :],
                            rhs=pr[:, 512 * j + zq : 512 * j + 512],
                            start=first,
                            stop=(stop and j == 3),
                            tile_position=(0, 64 * idx),
                            skip_group_check=True,
                        )

                for ei, (kt, mi, zq, w1) in enumerate(entries):
                    if fillers and (
                        ei % ration == 1 or (ration > 2 and ei >= nent - 4)
                    ):
                        pop_filler()
                    require(("k", g, kt // 8, (kt % 8) // 4))
                    require(("v", kt // 4))
                    kh, kcol = kt // 8, 128 * (kt % 8)
                    psa = ps_pool.tile([P, 1024], f32, tag="sc", bufs=2, name="psa")
                    psb = ps_pool.tile([P, 1024], f32, tag="sc", bufs=2, name="psb")
                    for j in range(4):
                        dst = (psa if j < 2 else psb)[:, 512 * (j % 2) + zq : 512 * (j % 2) + 512]
                        nc.tensor.matmul(
                            dst,
                            lhsT=kTh[g][kh][32 * j : 32 * j + 32, kcol : kcol + 128],
                            rhs=qTh[g][qh][32 * j : 32 * j + 32, qcol + zq : qcol + 512],
                            start=True,
                            stop=True,
                            tile_position=(32 * j, 0),
                        )
                    pr = probs_pool.tile([P, 2048], bf16, tag="pr", name="pr")
                    if zq == 0:
                        nc.scalar.activation(pr[:, :1024], psa[:], EXP)
                        nc.scalar.activation(pr[:, 1024:], psb[:], EXP)
                    else:
                        pra = pr[:, :1024].rearrange("p (t f) -> p t f", t=2)[:, :, zq:]
                        prb = pr[:, 1024:].rearrange("p (t f) -> p t f", t=2)[:, :, zq:]
                        psa3 = psa[:].rearrange("p (t f) -> p t f", t=2)[:, :, zq:]
                        psb3 = psb[:].rearrange("p (t f) -> p t f", t=2)[:, :, zq:]
                        nc.scalar.activation(pra, psa3, EXP)
                        nc.scalar.activation(prb, psb3, EXP)
                    if ei == min(1, nent - 1):
                        while pending:
                            pending.popleft()()
                        fillers.extend(post_fill)
                    if ei + 1 < nent:
                        nkt = entries[ei + 1][0]
                        require(("k", g, nkt // 8, (nkt % 8) // 4))
                        require(("v", nkt // 4))
                    if mi >= 0 and w1 > zq:
                        w = w1 - zq
                        pr3 = pr[:].rearrange("p (h f) -> p h f", h=4)[:, :, zq:w1]
                        emt = em_res[:, mi, :]
                        nc.vector.tensor_tensor(
                            pr3, pr3, emt[:, None, zq:w1].to_broadcast((P, 4, w)), MULT
                        )
                    # PV trails by one entry: by the time PV(e-1) issues, its
                    # exps are done and the next scores' PSUM WAR has aged out
                    if prev is not None:
                        emit_pv(prev)
                        if tail and ei - 1 == h0_ei and ei < nent - 1:
                            # first-half columns are final: copy them out now
                            # (DVE + ACT in parallel) so the tail compaction
                            # doesn't wait on them after the last entry
                            nc.vector.tensor_copy(sabA[:, :256], pvdn[:, :256])
                            nc.scalar.copy(sabB[:, :256], pvdn[:, 512:768])
                    prev = (pr, kt, ei == 0, zq)
                emit_pv(prev, stop=True)
                # eager pv->sbuf copy so the pv accumulator frees right away
                if tail:
                    if h0_ei >= nent - 2:
                        nc.vector.tensor_copy(sabA[:, :256], pvdn[:, :256])
                        nc.scalar.copy(sabB[:, :256], pvdn[:, 512:768])
                    nc.vector.tensor_copy(sabA[:, 256:], pvdn[:, 256:512])
                    nc.scalar.copy(sabB[:, 256:], pvdn[:, 768:1024])
                else:
                    nc.vector.tensor_copy(sabA[:], pvdn[:, :512])
                    nc.vector.tensor_copy(sabB[:], pvdn[:, 512:])
                return (g, qc, sabA, sabB)

            # ---- resource-tracked emission ----
            class Unit:
                __slots__ = ("fn", "done")

                def __init__(self, fn):
                    self.fn = fn
                    self.done = False

                def __call__(self):
                    if not self.done:
                        self.done = True
                        self.fn()

            done: set = set()
            queued: dict = {}
            fillers: deque = deque()
            pending: deque = deque()

            def resource_units(r):
                if r[0] == "v":
                    return v_quarter_units(r[1])
                w, g, half, c2 = r
                return qk_unit(w, g, half, c2)

            def needs(qc, g):
                res = []
                for kt, _, _, _ in plans[qc]:
                    res.append(("k", g, kt // 8, (kt % 8) // 4))
                res.append(("q", g, qc // 2, qc % 2))
                for kt, _, _, _ in plans[qc]:
                    res.append(("v", kt // 4))
                seen = []
                for r in res:
                    if r not in seen:
                        seen.append(r)
                return seen

            def require(r):
                if r in done:
                    return
                done.add(r)
                for u in queued.pop(r, None) or [Unit(f) for f in resource_units(r)]:
                    u()

            def queue(r):
                if r in done or r in queued:
                    return
                units = [Unit(f) for f in resource_units(r)]
                queued[r] = units
                fillers.extend(units)

            def pop_filler():
                while fillers:
                    u = fillers.popleft()
                    if not u.done:
                        u()
                        return

            startup_unit()
            done.update(
                {("q", 0, 0, 0), ("k", 0, 0, 0), ("q", 0, 0, 1), ("k", 0, 0, 1), ("v", 0)}
            )
            tail_args = None

            qc_order = list(range(NQC))
            # wo fillers deferred toward the later (larger) chunks, which have
            # the most attention entries to fill and the least projection work
            if NQC == 4:
                wo_sched = {(2, 0): 0, (3, 0): 1, (3, 1): 2}
            else:
                wo_sched = {(q, 0): q - 1 for q in range(1, NQC)}
            for qi, qc in enumerate(qc_order):
                for g in range(2):
                    require(("q", g, qc // 2, qc % 2))
                    # prefetch upcoming resources into the filler stream
                    post_fill = ()
                    if (qc, g) in wo_sched:
                        post_fill = [Unit(f) for f in wo_units(wo_sched[(qc, g)])]
                    if g == 0:
                        for r in needs(qc, 1):
                            queue(r)
                    elif qi + 1 < NQC:
                        for r in needs(qc_order[qi + 1], 0):
                            queue(r)
                        for r in needs(qc_order[qi + 1], 1):
                            queue(r)
                    # slower filler cadence in the final group: leftover units
                    # then flush right before the tail compaction, filling its
                    # otherwise-stalled dependency chain
                    last = qi == NQC - 1 and g == 1
                    cu = attention_g(qc, g, fillers, pending, post_fill,
                                     ration=3 if last else 2)
                    if cu is not None:
                        if last:
                            tail_args = cu
                        else:
                            pending.append(lambda a=cu: compact_unit(*a))
            while pending:
                pending.popleft()()
            while fillers:
                fillers.popleft()()
            if tail_args is not None:
                # wo(last qc) with kk=0 matmuls pre-started (attQ[0] is ready
                # well before the final compaction finishes attQ[1])
                lq = qc_order[-1]
                obs: dict = {}
                pres: dict = {}

                def wo_pre(sti):
                    obs[sti] = oout_pool.tile([P, D], bf16, tag="ob", name="ob")
                    ws = []
                    for n in range(2):
                        w = ps_pool.tile([P, 512], f32, tag="pj", bufs=2, name="wopt")
                        nc.tensor.matmul(
                            w[:],
                            lhsT=attQ[0][lq][:, 128 * sti : 128 * sti + 128],
                            rhs=wor_sb[:, 0, 512 * n : 512 * n + 512],
                            start=True, stop=False,
                        )
                        ws.append(w)
                    pres[sti] = ws

                def wo_fin(sti):
                    ob = obs[sti]
                    rows = slice(128 * (4 * lq + sti), 128 * (4 * lq + sti) + 128)
                    for n in range(2):
                        nc.tensor.matmul(
                            pres[sti][n][:],
                            lhsT=attQ[1][lq][:, 128 * sti : 128 * sti + 128],
                            rhs=wor_sb[:, 1, 512 * n : 512 * n + 512],
                            start=False, stop=True,
                        )
                        if n == 0:
                            nc.scalar.copy(ob[:, :512], pres[sti][n][:])
                        else:
                            nc.vector.tensor_copy(ob[:, 512:], pres[sti][n][:])
                    nc.sync.dma_start(out=out[rows, :], in_=ob[:])

                wo_pre(0)
                compact_half(*tail_args, 0)
                wo_fin(0)
                wo_pre(1)
                wo_fin(1)
                compact_half(*tail_args, 1)
                wo_pre(2)
                wo_fin(2)
                wo_pre(3)
                wo_fin(3)
            else:
                for u in wo_units(qc_order[-1], tail=True):
                    u()

    if not nc.is_finalized():
        nc.finalize()
    return nc


def _round_f32r(a):
    """Round fp32 array to the PE's f32r format (mantissa truncated to 11
    bits, round-to-nearest-even at bit 12)."""
    u = np.ascontiguousarray(a, dtype=np.float32).view(np.uint32)
    u2 = (u + np.uint32(0x7FF) + ((u >> np.uint32(12)) & np.uint32(1))) & np.uint32(0xFFFFF000)
    return u2.view(np.float32)


def _plan_from_mask(mask):
    """Classify [128, 512] tiles of exp(mask)^T; returns (plans, packed_tiles).

    Per tile: zq = leading fully-masked q-columns (even), w1 = end of the
    column window that still contains any masked element; columns >= w1 are
    all-ones and skip the multiply."""
    em = np.exp(mask.astype(np.float32))  # [q, k]
    emT = np.ascontiguousarray(em.T)  # [k, q]
    plans = []
    tiles = []
    tile_keys = {}
    for qc in range(NQC):
        ent = []
        for kt in range(NKT):
            t = emT[P * kt : P * (kt + 1), QC * qc : QC * (qc + 1)]
            if not t.any():
                continue  # fully masked out: skip tile entirely
            if (t == 1.0).all():
                ent.append((kt, -1, 0, 0))
                continue
            nz = np.flatnonzero(t.any(axis=0))
            zq = (int(nz[0]) // 2 * 2) if len(nz) else 0
            not_ones = np.flatnonzero(~(t == 1.0).all(axis=0))
            w1 = int(not_ones[-1]) + 1 if len(not_ones) else 0
            if w1 <= zq:
                ent.append((kt, -1, zq, 0))
                continue
            key = t.tobytes()
            mi = tile_keys.get(key)
            if mi is None:
                mi = len(tiles)
                tile_keys[key] = mi
                tiles.append(t.astype(ml_dtypes.bfloat16))
            ent.append((kt, mi, zq, w1))
        if ent and min(z for _, _, z, _ in ent) > 0:
            # every tile skips some leading columns -> those pv/dn columns
            # would never be written; disable skipping for this chunk
            ent = [(kt, mi, 0, w1) for kt, mi, _, w1 in ent]
        plans.append(tuple(ent))
    if tiles:
        packed = np.ascontiguousarray(np.stack(tiles))
    else:
        packed = np.zeros((1, P, QC), dtype=ml_dtypes.bfloat16)
    return tuple(plans), packed


def _sel_mats():
    s = np.zeros((4, P, P), dtype=np.float32)
    for m in range(32):
        s[0, m, m] = 1.0            # pvC rows 0-31   <- bankA rows 0-31
        s[0, m + 64, m + 32] = 1.0  # pvC rows 32-63  <- bankA rows 64-95
        s[1, m, m + 64] = 1.0       # pvC rows 64-95  <- bankB rows 0-31
        s[1, m + 64, m + 96] = 1.0  # pvC rows 96-127 <- bankB rows 64-95
    s[2, 32, 0:32] = 1.0            # dnC rows 0-31   <- bankA row 32
    s[2, 96, 32:64] = 1.0           # dnC rows 32-63  <- bankA row 96
    s[3, 32, 64:96] = 1.0           # dnC rows 64-95  <- bankB row 32
    s[3, 96, 96:128] = 1.0          # dnC rows 96-127 <- bankB row 96
    return s


def kernel(x, mask, section_log_len, wq, wk, wv, wo, seq_scale):
    from concourse.bass_utils import run_bass_kernel_spmd

    x = np.asarray(x, dtype=np.float32)
    assert x.shape == (B, S, D), x.shape
    mask2 = np.asarray(mask, dtype=np.float32).reshape(S, S)
    sll = np.asarray(section_log_len, dtype=np.float32).reshape(S)
    ss = np.asarray(seq_scale, dtype=np.float32).reshape(H)
    wq = np.asarray(wq, dtype=np.float32)
    wk = np.asarray(wk, dtype=np.float32)
    wv = np.asarray(wv, dtype=np.float32)
    wo = np.asarray(wo, dtype=np.float32)

    plans, tiles = _plan_from_mask(mask2)
    key = (plans, tiles.shape[0])
    nc = _GRAPH_CACHE.get(key)
    if nc is None:
        nc = _build_graph(plans, tiles.shape[0])
        _GRAPH_CACHE[key] = nc

    bf = ml_dtypes.bfloat16
    xT = [np.ascontiguousarray(x[b].T).astype(bf) for b in range(B)]
    selm = _sel_mats()
    sllB = np.ascontiguousarray(
        np.broadcast_to(sll[None, :], (P, S)), dtype=np.float32
    )

    in_maps = []
    for c in range(NCORES):
        b, g2 = divmod(c, 4)
        rows = slice(256 * g2, 256 * (g2 + 1))
        ssr = np.repeat(ss[8 * g2 : 8 * g2 + 8], HD) / math.sqrt(HD)
        in_maps.append(
            {
                "xT": xT[b],
                "aq": np.ascontiguousarray((wq[rows, :] * ssr[:, None]).T).astype(bf),
                "ak": np.ascontiguousarray(wk[rows, :].T).astype(bf),
                "av": np.ascontiguousarray(wv[rows, :].T).astype(bf),
                "wor": np.ascontiguousarray(wo[:, rows].T).astype(bf),
                "sllb": sllB,
                "emask": tiles,
                "sel": _round_f32r(selm),
            }
        )

    res = run_bass_kernel_spmd(nc, in_maps, core_ids=list(range(NCORES))).results
    out = np.zeros((B, S, D), dtype=np.float32)
    for c in range(NCORES):
        out[c // 4] += np.asarray(res[c]["out"], dtype=np.float32)
    return out
